# revision 7
# baseline (speedup 1.0000x reference)
"""Trainium2 Bass kernel for nn_Graph_to_Featuremaps_savemem.

Math: the reference computes, per batch b,
    scores[b,p,n] = (res @ nfr)[b,p] + (x @ nfh)[b,n]
    attn = softmax_n(scores);  out[b,p,c] = (attn @ (x @ W))[b,p,c]
Softmax over n is shift-invariant, so the (res @ nfr)[b,p] term cancels:
    attn[b,p,:] = softmax(x[b] @ nfh)   (independent of p)
    out[b,c,h,w] = relu(((softmax(x[b]@nfh) @ x[b]) @ W)[c])   broadcast over (h,w)
res_feature never affects the output. The kernel is therefore a tiny per-batch
compute (one 64-softmax + two small matmuls) followed by a broadcast write of
the (b,c)-constant planes — pure HBM-write-bound, sharded batch-parallel over
8 cores.

HBM write traffic is halved by emitting the output in fp16 (each plane is a
single rounded constant; norm rel-err ~1e-4, far inside the 2e-2 gate) and
upcasting to fp32 on the host during the unshard/gather step. Per core:
512 rows x 16384 cols x 2B = 16 MiB written.

The broadcast itself is done by the DMA engines, not compute: each (b,
c-half) row block has one physical (128, FILL_F) fp16 fill tile in SBUF, and
the output DMA uses a stride-0 middle dim in its source AP
([[part,128],[0,reps],[1,FILL_F]]) so each partition's fill row is re-read
to cover the 16384-wide output rows. The streaming phase runs at ~425 GB/s
(SBUF-fabric-limited), so the schedule is built to start it as early as
possible and keep both HWDGE rings primed:
  - inputs ship as two small tiles ([x^T|nfh], then [W]) so the s = x@nfh
    matmul starts before W has landed; both go on the SP ring, which also
    warms it. A dummy 256 B DMA warms the ACT ring during the prologue.
  - engine assignment keeps every stage off the busy engines: PE does
    s, per-batch sums (via a 0/1 selector), M = X@W and the four V = M^T e
    columns; ACT does exp and the M copy; DVE does only the reciprocal and
    the four fill broadcasts; GpSimd does all constant memsets and the
    1/sum partition-broadcast (InstPartitionBroadcast), keeping the PE free.
  - fill = (0 max V) * r[b] — one DVE tensor_scalar per block fuses relu
    and the softmax normalization, reading both fp32 scalars straight from
    PSUM/SBUF and writing the fp16 fill.
  - block 0's output DMA is split column-wise across both rings so both
    start streaming at the earliest possible moment.
"""

import numpy as np

N_CORES = 8
B, NODES, HID, C, H, W = 16, 64, 128, 256, 128, 128
HWP = H * W  # 16384
B_LOC = B // N_CORES  # 2 batches per core
FILL_F0 = 2048  # fill width for block 0 (fast start, proven 4 KiB descriptors)
FILL_F = 4096  # fill width for later blocks (8 KiB descriptors)

_NC_CACHE = {}


def build_nc():
    import concourse.bass as bass
    import concourse.bacc as bacc
    import concourse.mybir as mybir
    from concourse.tile import TileContext

    f32 = mybir.dt.float32
    f16 = mybir.dt.float16
    Alu = mybir.AluOpType
    Act = mybir.ActivationFunctionType

    nc = bacc.Bacc(None, target_bir_lowering=False, debug=False)
    inp1_d = nc.declare_dram_parameter("inp1", [128, 129], f32, isOutput=False)  # [x^T|nfh]
    inp2_d = nc.declare_dram_parameter("inp2", [128, C], f32, isOutput=False)  # W
    out_d = nc.declare_dram_parameter("out", [B_LOC * C, HWP], f16, isOutput=True)
    scr_d = nc.declare_dram_parameter("scr", [1, 64], f32, isOutput=True)

    def bcast_reps(ap, reps):
        # (128, F) AP -> (128, reps, F) AP re-reading each row reps x
        return type(ap)(ap.tensor, ap.offset, [list(ap.ap[0]), [0, reps], list(ap.ap[1])])

    with TileContext(nc) as tc:
        with (
            tc.tile_pool(name="singles", bufs=1) as singles,
            tc.tile_pool(name="fills", bufs=1) as fills,
            tc.tile_pool(name="psum", bufs=1, space="PSUM") as psum,
            tc.tile_pool(name="psumv", bufs=1, space="PSUM") as psumv,
        ):
            # ---- constants (no input deps; all on GpSimd to keep DVE free) ----
            SEL = singles.tile([128, 2], f32, tag="SEL")  # SEL[n,b] = [n//64 == b]
            nc.gpsimd.memset(SEL[:], 0.0)
            nc.gpsimd.memset(SEL[0:NODES, 0:1], 1.0)
            nc.gpsimd.memset(SEL[NODES : 2 * NODES, 1:2], 1.0)
            ZEROH = singles.tile([128, FILL_F], f16, tag="ZEROH")
            nc.gpsimd.memset(ZEROH[:], 0.0)

            # ---- warm the ACT HWDGE ring (SP ring is warmed by the inputs) ----
            nc.scalar.dma_start(out=scr_d[0:1, :], in_=SEL[0:64, 0:1])

            # ---- load inputs (two small DMAs, SP ring) ----
            INP1 = singles.tile([128, 129], f32, tag="INP1")
            nc.sync.dma_start(out=INP1[:], in_=inp1_d[:])
            INP2 = singles.tile([128, C], f32, tag="INP2")
            nc.sync.dma_start(out=INP2[:], in_=inp2_d[:])
            XT = INP1[:, 0:128]  # (hid, bn)
            NFH = INP1[:, 128:129]  # (hid, 1)
            Wt = INP2[:, :]  # (hid, c)

            # ---- e = exp(X @ nfh);  sums[b] = sum_b e ----
            s_ps = psum.tile([128, 1], f32, tag="s")
            nc.tensor.matmul(s_ps[:], XT, NFH)
            e_col = singles.tile([128, 1], f32, tag="e_col")
            nc.scalar.activation(e_col[:], s_ps[:], Act.Exp)
            sum_ps = psum.tile([1, 2], f32, tag="sum")
            nc.tensor.matmul(sum_ps[:], e_col[:], SEL[:])

            # ---- r = 1/sums (DVE), broadcast to all partitions (GpSimd) ----
            r_row = singles.tile([1, 2], f32, tag="r_row")
            nc.vector.reciprocal(r_row[:], sum_ps[:])
            RC = singles.tile([128, 2], f32, tag="RC")
            nc.gpsimd.partition_broadcast(RC[:], r_row[:])

            # ---- M = X @ W -> (bn, c) ----
            M_ps = psum.tile([128, C], f32, tag="M")
            nc.tensor.matmul(M_ps[:], XT, Wt)
            M_sb = singles.tile([128, C], f32, tag="M_sb")
            nc.scalar.activation(M_sb[:], M_ps[:], Act.Copy)

            for blk in range(4):
                b, hf = divmod(blk, 2)
                sl = slice(b * NODES, (b + 1) * NODES)
                # V'[b,hf] = M[b,:,hf-half]^T @ e[b] -> (128,1), c-major
                V_ps = psumv.tile([128, 1], f32, tag=f"V{blk}")
                nc.tensor.matmul(
                    V_ps[:], M_sb[sl, hf * 128 : (hf + 1) * 128], e_col[sl, :]
                )
                # fill[p, :] = relu(V'[p]) * r[b] = relu(V'[p]/sum_b), fp16
                ff = FILL_F0 if blk == 0 else FILL_F
                fill = fills.tile([128, ff], f16, tag=f"fill{blk}")
                nc.vector.tensor_scalar(
                    fill[:], ZEROH[:, 0:ff], V_ps[:], RC[:, b : b + 1],
                    op0=Alu.max, op1=Alu.mult,
                )
                r0 = blk * 128
                if blk == 0:
                    # split across both rings so both start streaming immediately
                    half = HWP // 2
                    nc.scalar.dma_start(
                        out=out_d[0:128, 0:half], in_=bcast_reps(fill[:], half // ff)
                    )
                    nc.sync.dma_start(
                        out=out_d[0:128, half:HWP], in_=bcast_reps(fill[:], half // ff)
                    )
                else:
                    eng = nc.scalar if blk == 2 else nc.sync
                    eng.dma_start(
                        out=out_d[r0 : r0 + 128, :], in_=bcast_reps(fill[:], HWP // ff)
                    )
    nc.finalize()
    return nc


def get_nc():
    if "nc" not in _NC_CACHE:
        _NC_CACHE["nc"] = build_nc()
    return _NC_CACHE["nc"]


def make_in_maps(input, node_fea_for_hidden, weight):
    x = np.asarray(input, np.float32)[0]  # (B, NODES, HID)
    nfh = np.asarray(node_fea_for_hidden, np.float32).reshape(HID, 1)
    w = np.ascontiguousarray(np.asarray(weight, np.float32))  # (HID, C)
    in_maps = []
    for i in range(N_CORES):
        xs = x[i * B_LOC : (i + 1) * B_LOC].reshape(B_LOC * NODES, HID)
        cat = np.concatenate([xs.T, nfh], axis=1)
        in_maps.append(
            {"inp1": np.ascontiguousarray(cat, np.float32), "inp2": w}
        )
    return in_maps


def run_spmd(in_maps, trace=False, **kw):
    from concourse.bass_utils import run_bass_kernel_spmd

    return run_bass_kernel_spmd(get_nc(), in_maps, list(range(N_CORES)), trace=trace, **kw)


def kernel(input, res_feature, node_fea_for_res, node_fea_for_hidden, weight):
    res = run_spmd(make_in_maps(input, node_fea_for_hidden, weight)).results
    out = np.concatenate(
        [r["out"].reshape(B_LOC, C, H, W) for r in res], axis=0
    )
    return out.astype(np.float32)
